# revision 2
# baseline (speedup 1.0000x reference)
"""Trainium2 Bass kernel for batched multi-head cross-attention (v3, shipped).

Problem: qkv (4, 1536, 3072) fp32, packed as 3*(8 heads * 64 ch) along dim 1.
Per (batch, head) item: S = (q*s)^T (k*s) -> softmax over key axis -> @ v.
bs*heads = 32 independent attention items sharded 4-per-core over 8 cores.

v3 design vs v2 baseline (all matmuls stay 16-bit; fp8 fails the 2e-2 gate):
  - G=2: S^T psum tiles are (128, 1024) = 2 banks, triple-buffered, and the
    MM2 accumulator is double-buffered (3*2 + 2 = 8 banks). Chunk boundaries
    no longer serialize on a single acc bank.
  - Normalization: denominator row copy on ACT, reciprocal_approx_fast on
    DVE, partition_broadcast on GPSIMD (Pool - otherwise idle), final
    multiply on DVE reading acc straight from PSUM. No PE broadcast matmul,
    no bc psum bank, no (64,512) DVE copies.
  - exp split ACT/DVE tuned per group (+1 optional column-split group) so
    both engines sit just below the PE's per-chunk time.

Per-core algorithm (per item, ch=64, T=3072):
  - q (pre-scaled by 1/sqrt(ch)), k as (128, T) fp16 SBUF tiles (dup halves
    for MM1 pair row-tiling).
  - vt (128, 24*65) bf16: per s-block v^T (128,64) + ones column (denominator).
  - For each 512-wide t-chunk, for each group g of 2 s-blocks:
      MM1 (PE):  S^T halves (128 s, 512 t) = k_blk.T @ q_chunk  [fp16 pair,
                 row-tiled on PE halves -> concurrent on HW]
      EXP:       w = exp(S^T) -> bf16 (ACT exact | DVE one-pass Schraudolph
                 tensor_scalar f32->int16 bf16-bit-pattern, <=3.3% err)
      MM2 (PE):  acc (65, 512) += vt_blk.T @ w_blk              [bf16]
    acc rows 0..63 = unnormalized output, row 64 = softmax denominator.
  - normalize: ACT stages denom row to SBUF, DVE reciprocal_approx_fast,
    GPSIMD partition_broadcast to 64 rows, DVE multiply vs acc PSUM, DMA out.

Softmax max-subtraction is skipped: S ~ N(0,1), exp stays in fp32/bf16 range,
and exp(x)/sum(exp(x)) is algebraically identical to the max-shifted form.
"""

import math
import os
import sys

import numpy as np

for _p in ("/opt/trn_rl_repo", "/opt/pypackages"):
    if os.path.isdir(_p) and _p not in sys.path:
        sys.path.append(_p)

import concourse.bass as bass
import concourse.mybir as mybir
import concourse.tile as tile
from concourse import bacc
from concourse.bass_utils import run_bass_kernel_spmd

N_CORES = 8
N_HEADS = 8
CH = 64  # head dim
F32 = mybir.dt.float32
F16 = mybir.dt.float16
BF16 = mybir.dt.bfloat16
I16 = mybir.dt.int16

MM_NP = np.float16  # q/k host dtype (MM1 operands)

TCHUNK = 512  # t columns per psum bank / matmul
SBLK = 128  # s rows per S^T block (psum partitions)
G = 2  # s-blocks per exp() group: free dim 1024 = 2 psum banks
CW = CH + 1  # vt block width (64 cols of v^T + ones column)

# Schraudolph-to-bf16-bits: bits16 = round(S*128/ln2 + (127*128 - 5.6))
# -> bitcast bf16 == 2^(S*log2e) with <=3.3% rel err (numerically optimized).
SCH_A = 128.0 / math.log(2.0)
SCH_B = 127.0 * 128.0 - 5.6

# Per-chunk exp engine assignment for the 12 groups: 'A' = ACT exact exp,
# 'D' = DVE Schraudolph, 'S' = split (ACT does block 2g, DVE block 2g+1 --
# halves the exp latency on the critical S-buffer recycle chain).
EXP_PATTERN = "ADADADADADAA"


def build_program(items: int, T: int, repeat: int = 1,
                  exp_pattern: str | None = None,
                  lookahead: int = 3,
                  sim_hw_pairs: bool = False,
                  stages: str = "full"):
    """Emit the per-core Bass program (SPMD across 8 cores).
    stages: 'mm1' | 'mm1exp' | 'nonorm' | 'full' - timing ablations."""
    do_exp = stages != "mm1"
    do_mm2 = stages in ("nonorm", "full")
    do_norm = stages == "full"
    if exp_pattern is None:
        exp_pattern = EXP_PATTERN
    SB = T // SBLK  # 24 s-blocks
    TC = T // TCHUNK  # 6 t-chunks
    NG = SB // G  # 12 groups per chunk
    GW = G * TCHUNK  # group width in psum/sbuf columns (1024)
    assert T % TCHUNK == 0 and T % SBLK == 0 and SB % G == 0
    assert len(exp_pattern) == NG

    nc = bacc.Bacc(
        "TRN2", target_bir_lowering=False, debug=False, num_devices=N_CORES
    )
    QP = 2 * CH  # q/k sbuf partition span (dup halves for pair row-tiling)
    qd = nc.dram_tensor("q", [items, CH, T], F16, kind="ExternalInput")
    kd = nc.dram_tensor("k", [items, CH, T], F16, kind="ExternalInput")
    vtd = nc.dram_tensor("vt", [items, SBLK, SB * CW], BF16, kind="ExternalInput")
    od = nc.dram_tensor("out", [items, CH, T], F32, kind="ExternalOutput")

    EXP = mybir.ActivationFunctionType.Exp

    with tile.TileContext(nc) as tc:
        with (
            tc.tile_pool(name="qkv", bufs=2) as qkpool,
            tc.tile_pool(name="w", bufs=3) as wpool,
            tc.tile_pool(name="osb", bufs=3) as opool,
            tc.tile_pool(name="rc", bufs=2) as rcpool,
            # PSUM budget (8 banks): S^T 3x2 + acc 2
            tc.tile_pool(name="spsum", bufs=3, space="PSUM") as spool,
            tc.tile_pool(name="accpsum", bufs=2, space="PSUM") as accpool,
        ):
            def emit_item(it):
                q_sb = qkpool.tile([QP, T], F16, tag="q")
                nc.sync.dma_start(q_sb[0:CH, :], qd[it])
                nc.sync.dma_start(q_sb[CH : 2 * CH, :], qd[it])
                k_sb = qkpool.tile([QP, T], F16, tag="k")
                nc.sync.dma_start(k_sb[0:CH, :], kd[it])
                nc.sync.dma_start(k_sb[CH : 2 * CH, :], kd[it])
                vt = qkpool.tile([SBLK, SB * CW], BF16, tag="vt")
                nc.sync.dma_start(vt[:], vtd[it])

                for tci in range(TC):
                    acc = accpool.tile([CW, TCHUNK], F32, tag="acc")
                    sts = {}
                    w_tiles = {}

                    def emit_mm1(g):
                        # One fp16 pair: s-blocks (2g, 2g+1) on PE row halves
                        # -> the two matmuls run concurrently on HW.
                        # sim_hw_pairs: halve the streamed columns so the
                        # cost model (which charges each matmul fully)
                        # approximates the HW-concurrent pair cost.
                        mc = TCHUNK // 2 if sim_hw_pairs else TCHUNK
                        st = spool.tile([SBLK, GW], F32, tag="s")
                        sts[g] = st
                        for half in range(G):
                            b = G * g + half
                            p0 = half * CH
                            nc.tensor.matmul(
                                st[:, half * TCHUNK : half * TCHUNK + mc],
                                lhsT=k_sb[p0 : p0 + CH, bass.ts(b, SBLK)],
                                rhs=q_sb[p0 : p0 + CH, bass.ts(tci, TCHUNK)][:, 0:mc],
                                start=True,
                                stop=True,
                            )

                    def _sch(dst_i16, src):
                        nc.vector.tensor_scalar(
                            dst_i16,
                            src,
                            SCH_A,
                            SCH_B,
                            mybir.AluOpType.mult,
                            mybir.AluOpType.add,
                        )

                    def emit_exp(g):
                        st = sts.pop(g)
                        mode = exp_pattern[g]
                        if mode == "D":
                            wi = wpool.tile([SBLK, GW], I16, tag="w")
                            _sch(wi[:], st[:])
                            w_tiles[g] = wi[:].bitcast(BF16)
                        elif mode == "S":
                            wi = wpool.tile([SBLK, GW], I16, tag="w")
                            nc.scalar.activation(
                                wi[:, 0:TCHUNK].bitcast(BF16),
                                st[:, 0:TCHUNK],
                                EXP,
                            )
                            _sch(wi[:, TCHUNK:GW], st[:, TCHUNK:GW])
                            w_tiles[g] = wi[:].bitcast(BF16)
                        else:
                            w = wpool.tile([SBLK, GW], BF16, tag="w")
                            nc.scalar.activation(w[:], st[:], EXP)
                            w_tiles[g] = w[:]

                    def emit_mm2(g):
                        w = w_tiles.pop(g)
                        for j in range(G):
                            b = G * g + j
                            nc.tensor.matmul(
                                acc[:],
                                lhsT=vt[:, b * CW : (b + 1) * CW],
                                rhs=w[:, bass.ts(j, TCHUNK)],
                                start=(b == 0),
                                stop=(b == SB - 1),
                                skip_group_check=True,
                            )

                    for g in range(min(lookahead, NG)):
                        emit_mm1(g)
                    for g in range(NG):
                        if do_exp:
                            emit_exp(g)
                        if g + lookahead < NG:
                            emit_mm1(g + lookahead)
                        if do_mm2:
                            emit_mm2(g)
                    if not do_norm:
                        continue

                    # normalization: evacuate the denom row via ACT (frees
                    # nothing but keeps DVE for the recip+mul), recip on DVE
                    # (approx_fast needs SBUF input), broadcast on GPSIMD,
                    # multiply on DVE vs acc PSUM, DMA out.
                    dn = rcpool.tile([1, TCHUNK], F32, tag="dn")
                    nc.scalar.copy(dn[:], acc[CH : CH + 1, :])
                    rcf = rcpool.tile([1, TCHUNK], F32, tag="rcf")
                    nc.vector.reciprocal_approx_fast(rcf[:], dn[:])
                    bcs = opool.tile([CH, TCHUNK], F32, tag="bcs")
                    nc.gpsimd.partition_broadcast(
                        bcs[:], rcf[0:1, :], channels=CH
                    )
                    osb = opool.tile([CH, TCHUNK], F32, tag="osb")
                    nc.vector.tensor_mul(osb[:], acc[0:CH, :], bcs[:])
                    nc.sync.dma_start(od[it][:, bass.ts(tci, TCHUNK)], osb[:])

            def body():
                for it in range(items):
                    emit_item(it)
                if not do_norm:
                    # ablation builds: keep the output tensor written
                    dummy = opool.tile([CH, TCHUNK], F32, tag="osb")
                    nc.vector.memset(dummy[:], 1.0)
                    nc.sync.dma_start(od[0][:, 0:TCHUNK], dummy[:])

            if repeat > 1:
                with tc.For_i(0, repeat, 1):
                    body()
            else:
                body()

    nc.compile()
    return nc


_CACHE: dict = {}


def _get_program(items: int, T: int):
    key = (items, T)
    if key not in _CACHE:
        _CACHE[key] = build_program(items, T)
    return _CACHE[key]


def _host_split(qkv: np.ndarray):
    """Split packed qkv into per-item q (pre-scaled) fp16, k fp16, and
    host-transposed vt bf16 (with ones columns), shapes per item:
    q,k (64, T); vt (128, SB*65)."""
    bs, width, T = qkv.shape
    ch = width // (3 * N_HEADS)
    n_items = bs * N_HEADS
    SB = T // SBLK
    q = qkv[:, : width // 3]
    k = qkv[:, width // 3 : 2 * (width // 3)]
    v = qkv[:, 2 * (width // 3) :]
    scale2 = np.float32(1.0 / math.sqrt(ch))  # (ch**-0.25)**2 folded into q
    qh = (q * scale2).reshape(n_items, ch, T).astype(MM_NP)
    kh = k.reshape(n_items, ch, T).astype(MM_NP)
    # vt[item, s_in_block, blk*65 + c] = v[item, c, blk*128 + s]; col 64 = 1
    vh = v.reshape(n_items, ch, SB, SBLK)
    vt = np.empty((n_items, SBLK, SB, CW), dtype=np.float32)
    vt[:, :, :, :ch] = vh.transpose(0, 3, 2, 1)
    vt[:, :, :, ch] = 1.0
    # f32 -> bf16 via round-to-nearest-even on the upper 16 bits
    u = vt.reshape(-1).view(np.uint32)
    u = (u + 0x7FFF + ((u >> 16) & 1)) >> 16
    vt16 = u.astype(np.uint16).view("<u2").reshape(n_items, SBLK, SB * CW)
    return qh, kh, vt16


def kernel(qkv, l):
    qkv = np.asarray(qkv, dtype=np.float32)
    l = int(l)
    bs, width, T = qkv.shape
    ch = width // (3 * N_HEADS)
    assert ch == CH, f"unexpected head dim {ch}"

    qh, kh, vt16 = _host_split(qkv)
    n_items = bs * N_HEADS
    ipc = n_items // N_CORES  # items per core

    nc = _get_program(ipc, T)
    in_maps = [
        {
            "q": np.ascontiguousarray(qh[c * ipc : (c + 1) * ipc]),
            "k": np.ascontiguousarray(kh[c * ipc : (c + 1) * ipc]),
            "vt": np.ascontiguousarray(vt16[c * ipc : (c + 1) * ipc]),
        }
        for c in range(N_CORES)
    ]
    res = run_bass_kernel_spmd(nc, in_maps, list(range(N_CORES)))
    agg = np.concatenate([res.results[c]["out"] for c in range(N_CORES)], axis=0)
    agg = agg.reshape(bs, N_HEADS * ch, T)
    return (agg[:, :, :l], agg[:, :, l : 2 * l], agg[:, :, 2 * l :])
